# revision 17
# baseline (speedup 1.0000x reference)
"""Trainium2 Bass kernel for BinarySplitDecoder (binary-tree leaf probabilities).

Contract: kernel(x) takes the FULL input x [65536, 1023] fp32 and returns the
FULL output [65536, 1024] fp32 (leaf probabilities of a depth-10 binary split
tree, level-major node ordering).

Sharding: pure data parallel — batch dim split evenly across 8 NeuronCores.

Strategy (fp16 + block layout; memory-bound, ~33.5 MB of HBM I/O per core):
  - Host casts x to fp16 and permutes columns (within each tree level, a
    bit-reversal involution); the device returns fp16 leaves in bit-reversed
    ("block") order, which the host un-permutes + casts back to fp32. The
    2e-2 relative-error gate makes fp16 safe (measured ~1.5e-3).
  - Block layout: each tree step writes left children into a packed lower
    half and right children into a packed upper half (instead of interleaving
    with stride 2). Packed 2-byte operands let every tensor_tensor run in the
    DVE 2x_1p perf mode — 2x throughput; the interleaved store of the fp32
    baseline forced 1x mode.
  - right = cur - left replaces cur * (1 - a): no separate (1 - x) pass.
  - Rows processed in chunks of g*128; partition p / free-group i holds batch
    row off + p*g + i, so every chunk DMA is one contiguous 2D block (one
    descriptor per partition — column-sliced 3D patterns cost ~6x more
    sequencer descriptor-gen time and stall the pipeline).
  - xin bufs=3: loads prefetch two chunks ahead. The framework hoists the
    next chunk's level-0 ops above the current chunk's deep levels in the
    in-order DVE queue, so a late load head-of-line-blocks ready work;
    2-deep prefetch left ~4 us bubbles at the ramp-to-steady transition.
  - The output store is split in halves: the left half (final after the
    level-9 multiply) drains while the subtract computes the right half.
  - Loads issue from the ACT sequencer (HWDGE), stores from SP: each
    sequencer drains in order, so a store's wait must not block loads.
  - Small chunks at both ends shorten the pipeline ramp and the final store
    drain. DVE (2x) and DMA both run ~95% of the steady window; remaining
    cost is fixed framework preamble/teardown (~18 us).
"""

import numpy as np

import concourse.bacc as bacc
import concourse.bass as bass
import concourse.mybir as mybir
from concourse.tile import TileContext
from concourse.bass_utils import run_bass_kernel_spmd

TREE_DEPTH = 10
N_NODES = (1 << TREE_DEPTH) - 1  # 1023
N_LEAVES = 1 << TREE_DEPTH  # 1024
N_CORES = 8
P = 128  # SBUF partitions
H = N_LEAVES // 2  # 512


def _bitrev(n: int, bits: int) -> int:
    r = 0
    for _ in range(bits):
        r = (r << 1) | (n & 1)
        n >>= 1
    return r


def _col_perm() -> np.ndarray:
    """xp[:, base+p] = x[:, base+rev_s(p)]: per-level bit-reversal so the
    block-layout walk consumes alphas from contiguous slices."""
    perm = np.arange(N_NODES)
    for s in range(TREE_DEPTH):
        base = (1 << s) - 1
        for p in range(1 << s):
            perm[base + p] = base + _bitrev(p, s)
    return perm


COL_PERM = _col_perm()
# block position j holds standard leaf rev(j); rev is an involution
OUT_PERM = np.array([_bitrev(m, TREE_DEPTH) for m in range(N_LEAVES)])


def build_nc(rows_per_core: int, G: int = 16) -> bass.Bass:
    """Per-core Bass program: DRAM "x" [rows_per_core, 1023] fp16 (columns
    pre-permuted) -> DRAM "y" [rows_per_core, 1024] fp16 (block leaf order).

    G must keep g*P divisible by 16 queues with balanced descriptor sizes:
    non-power-of-2 G measurably hotspots one DMA queue (g=14 left queue 15
    ~12 us busier than the rest, stalling a whole chunk's load).
    """
    units = rows_per_core // P
    # small chunks at both ends: short pipeline ramp AND short store drain
    chunks = [2, 4, 8] + [G] * ((units - 16) // G) + [2]
    assert sum(chunks) == units, (rows_per_core, chunks)
    f16 = mybir.dt.float16

    nc = bacc.Bacc("TRN2", target_bir_lowering=False, debug=False)
    x = nc.declare_dram_parameter("x", [rows_per_core, 511], f16, isOutput=False)
    xc_d = nc.declare_dram_parameter("xc", [rows_per_core, 512], f16, isOutput=False)
    y = nc.declare_dram_parameter("y", [rows_per_core, N_LEAVES], f16, isOutput=True)

    def x_view_cols(off, g, c0, c1):
        return x[off : off + g * P, c0:c1].rearrange("(p g) n -> p g n", g=g, p=P)

    def xc_view(off, g):
        return xc_d[off : off + g * P, :].rearrange("(p g) n -> p (g n)", g=g, p=P)

    def y_view(off, g, c0, c1):
        return y[off : off + g * P, c0:c1].rearrange("(p g) m -> p g m", g=g, p=P)

    with TileContext(nc) as tc:
        with (
            tc.tile_pool(name="xin", bufs=3) as xp,
            tc.tile_pool(name="xc", bufs=3) as xcp,
            tc.tile_pool(name="out", bufs=2) as outp,
            # bufs=2: with one buffer, chunk c+1's level-0 write must wait
            # for the level-9 reads of chunk c (WAR) — a per-chunk stall.
            tc.tile_pool(name="cur", bufs=2) as curp,
        ):
            off = 0
            for g in chunks:
                xt = xp.tile([P, g, 511], f16, tag="x")
                xct = xcp.tile([P, g, 512], f16, tag="xc")
                # column-split loads: DVE levels 0..7 only wait on the first
                # quarter of the chunk's bytes; level 8 on the next, level 9
                # on the rest — the tree walk starts ~4x earlier. The level-9
                # alphas live in their own DRAM array + tile (fully merged 2D
                # pattern, 16x fewer descriptors) and issue from the
                # otherwise-idle Pool sequencer.
                for c0, c1 in ((0, 255), (255, 511)):
                    nc.scalar.dma_start(
                        out=xt[:, :, c0:c1], in_=x_view_cols(off, g, c0, c1)
                    )
                nc.gpsimd.dma_start(out=xct[:], in_=xc_view(off, g))

                out_t = outp.tile([P, g, N_LEAVES], f16, tag="y")
                cur = None
                for d in range(TREE_DEPTH):
                    L = 1 << d
                    if d == TREE_DEPTH - 1:
                        # cur (the level-8 output) lives in out_t[:, H:]:
                        # left = cur * a9 into [0:H], then the subtract
                        # overwrites [H:] in place (per-element read
                        # precedes write on the DVE pipeline).
                        nxt = out_t
                        left = out_t[:, :, 0:H]
                        right = out_t[:, :, H:]
                    elif d == TREE_DEPTH - 2:
                        # level-8 output goes straight into the out tile's
                        # right half — frees the largest cur slot so xin
                        # affords 3 bufs within SBUF.
                        nxt = out_t
                        left = out_t[:, :, H : H + L]
                        right = out_t[:, :, H + L : H + 2 * L]
                    else:
                        # ping-pong intermediate levels between two shared
                        # slots (sized by the largest level using each tag)
                        nxt = curp.tile([P, g, 2 * L], f16, tag=f"cur{d % 2}")
                        left = nxt[:, :, 0:L]
                        right = nxt[:, :, L : 2 * L]
                    # [P, g, L] level-d alphas (level 9's live in xct)
                    a = (
                        xct[:, :, 0:L]
                        if d == TREE_DEPTH - 1
                        else xt[:, :, L - 1 : 2 * L - 1]
                    )
                    if d == 0:
                        nc.vector.tensor_copy(out=left, in_=a)
                        nc.vector.tensor_scalar(
                            out=right,
                            in0=a,
                            scalar1=-1.0,
                            scalar2=1.0,
                            op0=mybir.AluOpType.mult,
                            op1=mybir.AluOpType.add,
                        )
                    else:
                        nc.vector.tensor_mul(out=left, in0=cur, in1=a)
                        if d == TREE_DEPTH - 1:
                            # the left half of the leaves is final: start
                            # draining it while the right half is computed
                            nc.sync.dma_start(
                                out=y_view(off, g, 0, H), in_=out_t[:, :, 0:H]
                            )
                        nc.vector.tensor_tensor(
                            out=right, in0=cur, in1=left, op=mybir.AluOpType.subtract
                        )
                    if d == TREE_DEPTH - 2:
                        cur = out_t[:, :, H:]
                    else:
                        cur = nxt

                nc.sync.dma_start(
                    out=y_view(off, g, H, N_LEAVES), in_=out_t[:, :, H:]
                )
                off += g * P

    nc.compile()
    return nc


def _run(x: np.ndarray, **spmd_kwargs):
    """Shard x, run the Bass kernel on all 8 cores, return (y, BassKernelResults)."""
    x = np.asarray(x, dtype=np.float32)
    B = x.shape[0]
    assert B % N_CORES == 0 and x.shape[1] == N_NODES
    rows_per_core = B // N_CORES

    xh = x[:, COL_PERM].astype(np.float16)
    xab_h = np.ascontiguousarray(xh[:, :511])
    xc_h = np.ascontiguousarray(xh[:, 511:])

    nc = build_nc(rows_per_core)
    core_ids = list(range(N_CORES))
    in_maps = [
        {
            "x": xab_h[i * rows_per_core : (i + 1) * rows_per_core],
            "xc": xc_h[i * rows_per_core : (i + 1) * rows_per_core],
        }
        for i in core_ids
    ]
    res = run_bass_kernel_spmd(nc, in_maps, core_ids, **spmd_kwargs)
    out = np.concatenate([r["y"] for r in res.results], axis=0)
    out = out[:, OUT_PERM].astype(np.float32)
    return out, res


def kernel(x: np.ndarray) -> np.ndarray:
    return _run(x)[0]


# revision 21
# speedup vs baseline: 1.0067x; 1.0067x over previous
"""Trainium2 Bass kernel for BinarySplitDecoder (binary-tree leaf probabilities).

Contract: kernel(x) takes the FULL input x [65536, 1023] fp32 and returns the
FULL output [65536, 1024] fp32 (leaf probabilities of a depth-10 binary split
tree, level-major node ordering).

Sharding: pure data parallel — batch dim split evenly across 8 NeuronCores.

Strategy (fp16 + block layout; memory-bound, ~33.5 MB of HBM I/O per core):
  - Host casts x to fp16 and permutes columns (within each tree level, a
    bit-reversal involution); the device returns fp16 leaves in bit-reversed
    ("block") order, which the host un-permutes + casts back to fp32. The
    2e-2 relative-error gate makes fp16 safe (measured ~1.5e-3).
  - Block layout: each tree step writes left children into a packed lower
    half and right children into a packed upper half (instead of interleaving
    with stride 2). Packed 2-byte operands let every tensor_tensor run in the
    DVE 2x_1p perf mode — 2x throughput; the interleaved store of the fp32
    baseline forced 1x mode.
  - right = cur - left replaces cur * (1 - a): no separate (1 - x) pass.
  - Rows processed in chunks of g*128; partition p / free-group i holds batch
    row off + p*g + i, so every chunk DMA is one contiguous 2D block (one
    descriptor per partition — column-sliced 3D patterns cost ~6x more
    sequencer descriptor-gen time and stall the pipeline).
  - xin bufs=3: loads prefetch two chunks ahead. The framework hoists the
    next chunk's level-0 ops above the current chunk's deep levels in the
    in-order DVE queue, so a late load head-of-line-blocks ready work;
    2-deep prefetch left ~4 us bubbles at the ramp-to-steady transition.
  - The output store is split in halves: the left half (final after the
    level-9 multiply) drains while the subtract computes the right half.
  - Loads issue from the ACT sequencer (HWDGE), stores from SP: each
    sequencer drains in order, so a store's wait must not block loads.
  - Small chunks at both ends shorten the pipeline ramp and the final store
    drain. DVE (2x) and DMA both run ~95% of the steady window; remaining
    cost is fixed framework preamble/teardown (~18 us).
"""

import numpy as np

import concourse.bacc as bacc
import concourse.bass as bass
import concourse.mybir as mybir
from concourse.tile import TileContext
from concourse.bass_utils import run_bass_kernel_spmd

TREE_DEPTH = 10
N_NODES = (1 << TREE_DEPTH) - 1  # 1023
N_LEAVES = 1 << TREE_DEPTH  # 1024
N_CORES = 8
P = 128  # SBUF partitions
H = N_LEAVES // 2  # 512


def _bitrev(n: int, bits: int) -> int:
    r = 0
    for _ in range(bits):
        r = (r << 1) | (n & 1)
        n >>= 1
    return r


def _col_perm() -> np.ndarray:
    """xp[:, base+p] = x[:, base+rev_s(p)]: per-level bit-reversal so the
    block-layout walk consumes alphas from contiguous slices."""
    perm = np.arange(N_NODES)
    for s in range(TREE_DEPTH):
        base = (1 << s) - 1
        for p in range(1 << s):
            perm[base + p] = base + _bitrev(p, s)
    return perm


COL_PERM = _col_perm()
# block position j holds standard leaf rev(j); rev is an involution
OUT_PERM = np.array([_bitrev(m, TREE_DEPTH) for m in range(N_LEAVES)])


def build_nc(rows_per_core: int, G: int = 16) -> bass.Bass:
    """Per-core Bass program: DRAM "x" [rows_per_core, 1023] fp16 (columns
    pre-permuted) -> DRAM "y" [rows_per_core, 1024] fp16 (block leaf order).

    G must keep g*P divisible by 16 queues with balanced descriptor sizes:
    non-power-of-2 G measurably hotspots one DMA queue (g=14 left queue 15
    ~12 us busier than the rest, stalling a whole chunk's load).
    """
    units = rows_per_core // P
    # small chunks at both ends: short pipeline ramp AND short store drain
    chunks = [2, 4, 8] + [G] * ((units - 16) // G) + [2]
    assert sum(chunks) == units, (rows_per_core, chunks)
    f16 = mybir.dt.float16

    nc = bacc.Bacc("TRN2", target_bir_lowering=False, debug=False)
    xa_d = nc.declare_dram_parameter("xa", [rows_per_core, 255], f16, isOutput=False)
    xb_d = nc.declare_dram_parameter("xb", [rows_per_core, 256], f16, isOutput=False)
    xc_d = nc.declare_dram_parameter("xc", [rows_per_core, 512], f16, isOutput=False)
    y = nc.declare_dram_parameter("y", [rows_per_core, N_LEAVES], f16, isOutput=True)

    def in_view(t, off, g):
        return t[off : off + g * P, :].rearrange("(p g) n -> p (g n)", g=g, p=P)

    def y_view(off, g, c0, c1):
        return y[off : off + g * P, c0:c1].rearrange("(p g) m -> p g m", g=g, p=P)

    with TileContext(nc) as tc:
        with (
            tc.tile_pool(name="xa", bufs=3) as xap,
            tc.tile_pool(name="xb", bufs=3) as xbp,
            tc.tile_pool(name="xc", bufs=3) as xcp,
            tc.tile_pool(name="out", bufs=2) as outp,
            # bufs=2: with one buffer, chunk c+1's level-0 write must wait
            # for the level-9 reads of chunk c (WAR) — a per-chunk stall.
            tc.tile_pool(name="cur", bufs=2) as curp,
        ):
            off = 0
            for g in chunks:
                # column-split loads from three separate contiguous DRAM
                # arrays (levels 0-7 / 8 / 9 alphas): DVE starts the tree
                # walk after ~25% of a chunk's bytes, every DMA is a fully
                # merged 2D pattern (one descriptor per partition — sliced
                # 3D patterns cost ~7x more sequencer descriptor-gen time
                # and delay later loads), and 3 bufs lets loads prefetch
                # two chunks ahead. All on the hardware-DGE ACT sequencer —
                # the Pool sequencer's software DGE is far too slow.
                xat = xap.tile([P, g, 255], f16, tag="xa")
                xbt = xbp.tile([P, g, 256], f16, tag="xb")
                xct = xcp.tile([P, g, 512], f16, tag="xc")
                nc.scalar.dma_start(out=xat[:], in_=in_view(xa_d, off, g))
                nc.scalar.dma_start(out=xbt[:], in_=in_view(xb_d, off, g))
                nc.scalar.dma_start(out=xct[:], in_=in_view(xc_d, off, g))

                out_t = outp.tile([P, g, N_LEAVES], f16, tag="y")
                cur = None
                for d in range(TREE_DEPTH):
                    L = 1 << d
                    if d == TREE_DEPTH - 1:
                        # cur (the level-8 output) lives in out_t[:, H:]:
                        # left = cur * a9 into [0:H], then the subtract
                        # overwrites [H:] in place (per-element read
                        # precedes write on the DVE pipeline).
                        nxt = out_t
                        left = out_t[:, :, 0:H]
                        right = out_t[:, :, H:]
                    elif d == TREE_DEPTH - 2:
                        # level-8 output goes straight into the out tile's
                        # right half — frees the largest cur slot so xin
                        # affords 3 bufs within SBUF.
                        nxt = out_t
                        left = out_t[:, :, H : H + L]
                        right = out_t[:, :, H + L : H + 2 * L]
                    else:
                        # ping-pong intermediate levels between two shared
                        # slots (sized by the largest level using each tag)
                        nxt = curp.tile([P, g, 2 * L], f16, tag=f"cur{d % 2}")
                        left = nxt[:, :, 0:L]
                        right = nxt[:, :, L : 2 * L]
                    # [P, g, L] level-d alphas
                    if d == TREE_DEPTH - 1:
                        a = xct[:, :, 0:L]
                    elif d == TREE_DEPTH - 2:
                        a = xbt[:, :, 0:L]
                    else:
                        a = xat[:, :, L - 1 : 2 * L - 1]
                    if d == 0:
                        nc.vector.tensor_copy(out=left, in_=a)
                        nc.vector.tensor_scalar(
                            out=right,
                            in0=a,
                            scalar1=-1.0,
                            scalar2=1.0,
                            op0=mybir.AluOpType.mult,
                            op1=mybir.AluOpType.add,
                        )
                    else:
                        nc.vector.tensor_mul(out=left, in0=cur, in1=a)
                        if d == TREE_DEPTH - 1:
                            # the left half of the leaves is final: start
                            # draining it while the right half is computed
                            nc.sync.dma_start(
                                out=y_view(off, g, 0, H), in_=out_t[:, :, 0:H]
                            )
                        nc.vector.tensor_tensor(
                            out=right, in0=cur, in1=left, op=mybir.AluOpType.subtract
                        )
                    if d == TREE_DEPTH - 2:
                        cur = out_t[:, :, H:]
                    else:
                        cur = nxt

                nc.sync.dma_start(
                    out=y_view(off, g, H, N_LEAVES), in_=out_t[:, :, H:]
                )
                off += g * P

    nc.compile()
    return nc


def _run(x: np.ndarray, **spmd_kwargs):
    """Shard x, run the Bass kernel on all 8 cores, return (y, BassKernelResults)."""
    x = np.asarray(x, dtype=np.float32)
    B = x.shape[0]
    assert B % N_CORES == 0 and x.shape[1] == N_NODES
    rows_per_core = B // N_CORES

    xh = x[:, COL_PERM].astype(np.float16)
    xa_h = np.ascontiguousarray(xh[:, :255])
    xb_h = np.ascontiguousarray(xh[:, 255:511])
    xc_h = np.ascontiguousarray(xh[:, 511:])

    nc = build_nc(rows_per_core)
    core_ids = list(range(N_CORES))
    in_maps = [
        {
            "xa": xa_h[i * rows_per_core : (i + 1) * rows_per_core],
            "xb": xb_h[i * rows_per_core : (i + 1) * rows_per_core],
            "xc": xc_h[i * rows_per_core : (i + 1) * rows_per_core],
        }
        for i in core_ids
    ]
    res = run_bass_kernel_spmd(nc, in_maps, core_ids, **spmd_kwargs)
    out = np.concatenate([r["y"] for r in res.results], axis=0)
    out = out[:, OUT_PERM].astype(np.float32)
    return out, res


def kernel(x: np.ndarray) -> np.ndarray:
    return _run(x)[0]


# revision 23
# speedup vs baseline: 1.1920x; 1.1841x over previous
"""Trainium2 Bass kernel for BinarySplitDecoder (binary-tree leaf probabilities).

Contract: kernel(x) takes the FULL input x [65536, 1023] fp32 and returns the
FULL output [65536, 1024] fp32 (leaf probabilities of a depth-10 binary split
tree, level-major node ordering).

Sharding: pure data parallel — batch dim split evenly across 8 NeuronCores.

Strategy (fp16 + block layout; memory-bound, ~33.5 MB of HBM I/O per core):
  - Host casts x to fp16 and permutes columns (within each tree level, a
    bit-reversal involution); the device returns fp16 leaves in bit-reversed
    ("block") order, which the host un-permutes + casts back to fp32. The
    2e-2 relative-error gate makes fp16 safe (measured ~1.5e-3).
  - Block layout: each tree step writes left children into a packed lower
    half and right children into a packed upper half (instead of interleaving
    with stride 2). Packed 2-byte operands let every tensor_tensor run in the
    DVE 2x_1p perf mode — 2x throughput; the interleaved store of the fp32
    baseline forced 1x mode. (Measured: all tree ops run at ~1.85 elem/ns
    per partition = 2x; keeping one wide xt tile matters — separate small
    alpha tiles made every DVE op ~20% slower.)
  - right = cur - left replaces cur * (1 - a): no separate (1 - x) pass.
  - Rows processed in chunks of g*128; partition p / free-group i holds batch
    row off + p*g + i. Chunk loads split into three column pieces (levels
    0-7 / 8 / 9): the tree walk starts after ~25% of the chunk's bytes.
    Piece A of chunk c+1 is issued BEFORE pieces B/C of chunk c, giving the
    next chunk's first bytes a full chunk of extra lead time (the framework
    hoists chunk c+1's level-0 ops above chunk c's deep levels in the
    in-order DVE queue, so a late piece A head-of-line-blocks ready work).
  - xin bufs=3 (loads prefetch two chunks ahead); SBUF affords this because
    the level-8 output goes straight into the out tile's right half and the
    level-9 subtract runs in place on it, freeing the largest cur slot.
  - The output store is split in halves: the left half (final after the
    level-9 multiply) drains while the subtract computes the right half.
  - Loads issue from the ACT sequencer (HWDGE), stores from SP: each
    sequencer drains in order, so a store's wait must not block loads.
    (Pool-sequencer DMA is software-DGE — far too slow for bulk loads.)
  - Small chunks at both ends shorten the pipeline ramp and the final store
    drain. G must be a power of two: g=14 hotspotted one DMA queue ~12 us.
  - DVE (2x) and DMA both run ~95% of the steady window; remaining cost is
    fixed framework preamble/teardown (~18 us).
"""

import numpy as np

import concourse.bacc as bacc
import concourse.bass as bass
import concourse.mybir as mybir
from concourse.tile import TileContext
from concourse.bass_utils import run_bass_kernel_spmd

TREE_DEPTH = 10
N_NODES = (1 << TREE_DEPTH) - 1  # 1023
N_LEAVES = 1 << TREE_DEPTH  # 1024
N_CORES = 8
P = 128  # SBUF partitions
H = N_LEAVES // 2  # 512
PIECES = ((0, 255), (255, 511), (511, 1023))  # levels 0-7 / 8 / 9 alphas


def _bitrev(n: int, bits: int) -> int:
    r = 0
    for _ in range(bits):
        r = (r << 1) | (n & 1)
        n >>= 1
    return r


def _col_perm() -> np.ndarray:
    """xp[:, base+p] = x[:, base+rev_s(p)]: per-level bit-reversal so the
    block-layout walk consumes alphas from contiguous slices."""
    perm = np.arange(N_NODES)
    for s in range(TREE_DEPTH):
        base = (1 << s) - 1
        for p in range(1 << s):
            perm[base + p] = base + _bitrev(p, s)
    return perm


COL_PERM = _col_perm()
# block position j holds standard leaf rev(j); rev is an involution
OUT_PERM = np.array([_bitrev(m, TREE_DEPTH) for m in range(N_LEAVES)])


def build_nc(rows_per_core: int, G: int = 16) -> bass.Bass:
    """Per-core Bass program: DRAM "x" [rows_per_core, 1023] fp16 (columns
    pre-permuted) -> DRAM "y" [rows_per_core, 1024] fp16 (block leaf order).
    """
    units = rows_per_core // P
    # small chunks at both ends: short pipeline ramp AND short store drain
    chunks = [2, 4, 8] + [G] * ((units - 16) // G) + [2]
    assert sum(chunks) == units, (rows_per_core, chunks)
    offs = np.concatenate([[0], np.cumsum(chunks)[:-1]]) * P
    f16 = mybir.dt.float16

    nc = bacc.Bacc("TRN2", target_bir_lowering=False, debug=False)
    x = nc.declare_dram_parameter("x", [rows_per_core, N_NODES], f16, isOutput=False)
    y = nc.declare_dram_parameter("y", [rows_per_core, N_LEAVES], f16, isOutput=True)

    def x_view(off, g, c0, c1):
        return x[off : off + g * P, c0:c1].rearrange("(p g) n -> p g n", g=g, p=P)

    def y_view(off, g, c0, c1):
        return y[off : off + g * P, c0:c1].rearrange("(p g) m -> p g m", g=g, p=P)

    with TileContext(nc) as tc:
        with (
            tc.tile_pool(name="xin", bufs=3) as xp,
            tc.tile_pool(name="out", bufs=2) as outp,
            # bufs=2: with one buffer, chunk c+1's level-0 write must wait
            # for the level-9 reads of chunk c (WAR) — a per-chunk stall.
            tc.tile_pool(name="cur", bufs=2) as curp,
        ):
            xts = {}

            def load_piece(c, i):
                if c >= len(chunks):
                    return
                if c not in xts:
                    xts[c] = xp.tile(
                        [P, chunks[c], N_NODES], f16, tag="x", name=f"xt{c}"
                    )
                c0, c1 = PIECES[i]
                nc.scalar.dma_start(
                    out=xts[c][:, :, c0:c1],
                    in_=x_view(int(offs[c]), chunks[c], c0, c1),
                )

            load_piece(0, 0)
            for c, g in enumerate(chunks):
                off = int(offs[c])
                # piece A of the NEXT chunk goes first in the ACT queue
                load_piece(c + 1, 0)
                load_piece(c, 1)
                load_piece(c, 2)
                xt = xts.pop(c)

                out_t = outp.tile([P, g, N_LEAVES], f16, tag="y")
                cur = None
                for d in range(TREE_DEPTH):
                    L = 1 << d
                    if d == TREE_DEPTH - 1:
                        # cur (the level-8 output) lives in out_t[:, H:]:
                        # left = cur * a9 into [0:H], then the subtract
                        # overwrites [H:] in place (per-element read
                        # precedes write on the DVE pipeline).
                        left = out_t[:, :, 0:H]
                        right = out_t[:, :, H:]
                    elif d == TREE_DEPTH - 2:
                        # level-8 output goes straight into the out tile's
                        # right half — frees the largest cur slot so xin
                        # affords 3 bufs within SBUF.
                        left = out_t[:, :, H : H + L]
                        right = out_t[:, :, H + L : H + 2 * L]
                    else:
                        # ping-pong intermediate levels between two shared
                        # slots (sized by the largest level using each tag)
                        nxt = curp.tile([P, g, 2 * L], f16, tag=f"cur{d % 2}")
                        left = nxt[:, :, 0:L]
                        right = nxt[:, :, L : 2 * L]
                    a = xt[:, :, L - 1 : 2 * L - 1]  # [P, g, L] level-d alphas
                    if d == 0:
                        nc.vector.tensor_copy(out=left, in_=a)
                        nc.vector.tensor_scalar(
                            out=right,
                            in0=a,
                            scalar1=-1.0,
                            scalar2=1.0,
                            op0=mybir.AluOpType.mult,
                            op1=mybir.AluOpType.add,
                        )
                    else:
                        nc.vector.tensor_mul(out=left, in0=cur, in1=a)
                        if d == TREE_DEPTH - 1:
                            # the left half of the leaves is final: start
                            # draining it while the right half is computed
                            nc.sync.dma_start(
                                out=y_view(off, g, 0, H), in_=out_t[:, :, 0:H]
                            )
                        nc.vector.tensor_tensor(
                            out=right, in0=cur, in1=left, op=mybir.AluOpType.subtract
                        )
                    if d >= TREE_DEPTH - 2:
                        cur = out_t[:, :, H:]
                    else:
                        cur = nxt

                nc.sync.dma_start(
                    out=y_view(off, g, H, N_LEAVES), in_=out_t[:, :, H:]
                )

    nc.compile()
    return nc


def _run(x: np.ndarray, **spmd_kwargs):
    """Shard x, run the Bass kernel on all 8 cores, return (y, BassKernelResults)."""
    x = np.asarray(x, dtype=np.float32)
    B = x.shape[0]
    assert B % N_CORES == 0 and x.shape[1] == N_NODES
    rows_per_core = B // N_CORES

    xh = np.ascontiguousarray(x[:, COL_PERM].astype(np.float16))

    nc = build_nc(rows_per_core)
    core_ids = list(range(N_CORES))
    in_maps = [
        {"x": xh[i * rows_per_core : (i + 1) * rows_per_core]} for i in core_ids
    ]
    res = run_bass_kernel_spmd(nc, in_maps, core_ids, **spmd_kwargs)
    out = np.concatenate([r["y"] for r in res.results], axis=0)
    out = out[:, OUT_PERM].astype(np.float32)
    return out, res


def kernel(x: np.ndarray) -> np.ndarray:
    return _run(x)[0]
